# revision 1
# baseline (speedup 1.0000x reference)
"""MAE ViT (PrawnMKPModel) forward pass on 8 TRN2 NeuronCores.

Strategy: pure data parallelism (8 samples/core). All ragged gather/pad/
scatter work is hoisted to host-side input prep by exploiting permutation
equivariance of attention: the decoder runs in *permuted* token order
(visible tokens first, masked after), so the device kernel is a fully
dense transformer. Host folds LN affines into adjacent matmul weights,
pre-scales Q by 1/sqrt(dh), and un-permutes the output at the end.

Device pipeline per core (8 samples):
  patch-embed matmul -> 6 encoder blocks (seq 50, pad 64/sample, D=384)
  -> dec-embed -> 4 decoder blocks (seq 196 = 2x98 tiles/sample, DD=256)
  -> pred head. Softmax is computed un-normalized (scores are small, no
  max-subtraction needed); the denominator Z comes for free from an
  appended ones-column on V, and 1/Z is applied to the token-major AV
  output where it is a native per-partition scale.
"""

import os
import sys

import numpy as np

for _p in ("/opt/trn_rl_repo", "/root/.axon_site/_ro/trn_rl_repo"):
    if os.path.isdir(_p) and _p not in sys.path:
        sys.path.append(_p)

import ml_dtypes  # noqa: E402
import concourse.bass as bass  # noqa: E402
import concourse.mybir as mybir  # noqa: E402
import concourse.tile as tile  # noqa: E402
from concourse import bacc  # noqa: E402
from concourse.bass_utils import run_bass_kernel_spmd  # noqa: E402
from concourse.masks import make_identity  # noqa: E402

F32 = mybir.dt.float32
BF16 = mybir.dt.bfloat16
AF = mybir.ActivationFunctionType

B, C, H, W = 64, 3, 224, 224
P, NP, D, HEADS, DEPTH = 16, 196, 384, 6, 6
DD, DHEADS, DDEPTH = 256, 8, 4
NVIS = 49
DH = D // HEADS          # 64
DDH = DD // DHEADS       # 32
NCORES = 8
BS = B // NCORES         # 8 samples per core
PATCH = C * P * P        # 768
EPS = 1e-5

# encoder token layout: densely packed 50/sample -> 400 tokens,
# tiles of [128, 128, 128, 16]
ET = 50 * BS             # 400
ECH = 4
ESZ = [128, 128, 128, 16]
EOFF = [0, 128, 256, 384]
EKC = D // 128           # 3
EFH = 4 * D              # 1536
# decoder token layout: 196/sample = 2x98 tiles -> 16x[98] tiles
DT = NP * BS             # 1568
DTILES = 2 * BS          # 16
DKC = DD // 128          # 2
DFH = 4 * DD             # 1024
DNCH = 392               # linear N-chunk width for decoder (4 chunks)

_CACHE = {}


def _bf(x):
    return np.ascontiguousarray(np.asarray(x, np.float32).astype(ml_dtypes.bfloat16))


def _f32(x):
    return np.ascontiguousarray(np.asarray(x, np.float32))


def _build_program():
    nc = bacc.Bacc()

    # ---- DRAM parameters (per core) ----
    xg_t = nc.declare_dram_parameter("xg_t", [PATCH, 50 * BS], BF16, isOutput=False)
    bias_tok = nc.declare_dram_parameter("bias_tok", [ET, D], F32, isOutput=False)
    dpe = nc.declare_dram_parameter("dpe", [DT, DD], F32, isOutput=False)
    wconv_t = nc.declare_dram_parameter("wconv_t", [PATCH, D], BF16, isOutput=False)
    e_qkvw = nc.declare_dram_parameter("e_qkvw", [DEPTH, D, 3 * D], BF16, isOutput=False)
    e_ow = nc.declare_dram_parameter("e_ow", [DEPTH, D, D], BF16, isOutput=False)
    e_w1 = nc.declare_dram_parameter("e_w1", [DEPTH, D, EFH], BF16, isOutput=False)
    e_w2 = nc.declare_dram_parameter("e_w2", [DEPTH, EFH, D], BF16, isOutput=False)
    d_qkvw = nc.declare_dram_parameter("d_qkvw", [DDEPTH, DD, 3 * DD], BF16, isOutput=False)
    d_ow = nc.declare_dram_parameter("d_ow", [DDEPTH, DD, DD], BF16, isOutput=False)
    d_w1 = nc.declare_dram_parameter("d_w1", [DDEPTH, DD, DFH], BF16, isOutput=False)
    d_w2 = nc.declare_dram_parameter("d_w2", [DDEPTH, DFH, DD], BF16, isOutput=False)
    dew = nc.declare_dram_parameter("dew", [D, DD], BF16, isOutput=False)
    predw = nc.declare_dram_parameter("predw", [DD, P * P * C], BF16, isOutput=False)
    out = nc.declare_dram_parameter("out", [DT, P * P * C], F32, isOutput=True)

    with tile.TileContext(nc) as tc:
        with (
            tc.tile_pool(name="persist", bufs=1) as persist,
            tc.tile_pool(name="spool", bufs=8) as spool,
            tc.tile_pool(name="psb", bufs=2, space="PSUM") as psb,
            tc.tile_pool(name="psS", bufs=2, space="PSUM") as psS,
            tc.tile_pool(name="psO", bufs=2, space="PSUM") as psO,
            tc.tile_pool(name="ptr", bufs=2, space="PSUM") as ptr,
        ):
            ident = persist.tile([128, 128], BF16, tag="ident", name="ident")
            make_identity(nc, ident[:])
            eps_t = persist.tile([128, 1], F32, tag="eps", name="eps")
            nc.gpsimd.memset(eps_t[:], EPS)

            def ln_phase(x_list, sizes, emit):
                """Batched LN over the tiles: one Sqrt instruction per phase.

                emit(t, mb_ap, ri_ap) applies the normalize for tile t.
                """
                nt = len(x_list)
                rows = max(sizes)
                mvp = spool.tile([rows, nt, 2], F32, tag="mvp", name="mvp", bufs=3)
                if min(sizes) < rows:
                    nc.vector.memset(mvp[:], 1.0)
                for t in range(nt):
                    st6 = spool.tile([rows, 6], F32, tag="st6", name="st6")
                    nc.vector.bn_stats(st6[:sizes[t]], x_list[t][:])
                    nc.vector.bn_aggr(mvp[:sizes[t], t, :], st6[:sizes[t]])
                sd = spool.tile([rows, nt], F32, tag="sd", name="sd", bufs=3)
                nc.scalar.activation(sd[:], mvp[:, :, 1], AF.Sqrt, bias=eps_t[:rows])
                ri = spool.tile([rows, nt], F32, tag="ri", name="ri", bufs=3)
                nc.vector.reciprocal(ri[:], sd[:])
                mb = spool.tile([rows, nt], F32, tag="mb", name="mb", bufs=3)
                nc.vector.scalar_tensor_tensor(
                    mb[:], mvp[:, :, 0], -1.0, ri[:],
                    mybir.AluOpType.mult, mybir.AluOpType.mult)
                for t in range(nt):
                    emit(t, mb[:sizes[t], t:t + 1], ri[:sizes[t], t:t + 1])

            def pack_transpose(srcs, dsts, sizes, gsz, maxw=1024):
                """dsts[f][:, off_t:off_t+sizes[t]] = srcs[t][:, 128f:128(f+1)].T

                Groups gsz transposes into one PSUM bank, evicted by a single
                wide DVE copy. sizes[t] = partition count of srcs[t].
                """
                nt, nf = len(srcs), len(dsts)
                offs = [sum(sizes[:t]) for t in range(nt + 1)]
                for f in range(nf):
                    for g0 in range(0, nt, gsz):
                        g = min(gsz, nt - g0)
                        w = offs[g0 + g] - offs[g0]
                        pt = ptr.tile([128, maxw], BF16, tag="ptr", name="ptr")
                        for k in range(g):
                            r = sizes[g0 + k]
                            o = offs[g0 + k] - offs[g0]
                            nc.tensor.transpose(
                                pt[:128, o:o + r],
                                srcs[g0 + k][:, 128 * f:128 * (f + 1)],
                                ident[:r, :r])
                        nc.vector.tensor_copy(
                            dsts[f][:, offs[g0]:offs[g0 + g]], pt[:128, :w])

            # ================= ENCODER =================
            x_e = [persist.tile([ESZ[c], D], F32, tag=f"x_e{c}", name=f"x_e{c}")
                   for c in range(ECH)]
            x_d = [persist.tile([98, DD], F32, tag=f"x_d{t}", name=f"x_d{t}")
                   for t in range(DTILES)]

            with (
                tc.tile_pool(name="ewpool", bufs=2) as wpool,
                tc.tile_pool(name="eapool", bufs=1) as apool,
            ):
                # patch embed
                xg_sb = apool.tile([128, PATCH // 128, 50 * BS], BF16, tag="xg",
                                   name="xg")
                nc.gpsimd.dma_start(
                    xg_sb[:], xg_t[:].rearrange("(kc p) t -> p kc t", p=128))
                wc_sb = apool.tile([128, PATCH // 128, D], BF16, tag="wc", name="wc")
                nc.gpsimd.dma_start(
                    wc_sb[:], wconv_t[:].rearrange("(kc p) f -> p kc f", p=128))
                bias_sb = [apool.tile([ESZ[c], D], F32, tag=f"btok{c}",
                                      name=f"btok{c}") for c in range(ECH)]
                for c in range(ECH):
                    nc.gpsimd.dma_start(bias_sb[c][:],
                                        bias_tok[EOFF[c]:EOFF[c] + ESZ[c], :])
                for c in range(ECH):
                    ps = psb.tile([ESZ[c], D], F32, tag="psbig", name="psbig")
                    for kc in range(PATCH // 128):
                        nc.tensor.matmul(ps[:],
                                         xg_sb[:, kc, EOFF[c]:EOFF[c] + ESZ[c]],
                                         wc_sb[:, kc, :],
                                         start=(kc == 0),
                                         stop=(kc == PATCH // 128 - 1))
                    nc.vector.tensor_add(x_e[c][:], bias_sb[c][:], ps[:])

                def enc_layer(i):
                    qkvw_sb = wpool.tile([128, EKC, 3 * D], BF16, tag="eqkvw",
                                         name="eqkvw")
                    nc.gpsimd.dma_start(qkvw_sb[:],
                                      e_qkvw[i].rearrange("(kc p) f -> p kc f", p=128))
                    ow_sb = wpool.tile([128, EKC, D], BF16, tag="eow", name="eow")
                    nc.gpsimd.dma_start(ow_sb[:],
                                      e_ow[i].rearrange("(kc p) f -> p kc f", p=128))
                    w1_sb = wpool.tile([128, EKC, EFH], BF16, tag="ew1", name="ew1")
                    nc.gpsimd.dma_start(w1_sb[:],
                                      e_w1[i].rearrange("(kc p) f -> p kc f", p=128))
                    w2_sb = wpool.tile([128, EFH // 128, D], BF16, tag="ew2",
                                       name="ew2")
                    nc.gpsimd.dma_start(w2_sb[:],
                                      e_w2[i].rearrange("(kc p) f -> p kc f", p=128))

                    # LN1 + transpose
                    xT = [apool.tile([128, ET], BF16, tag=f"exT{f}", name=f"exT{f}", bufs=2)
                          for f in range(EKC)]
                    xhs = [apool.tile([ESZ[c], D], BF16, tag=f"exh{c}", name=f"exh{c}",
                                      bufs=2) for c in range(ECH)]

                    def emit_ln1(t, mb, ri, xhs=xhs):
                        nc.scalar.activation(xhs[t][:], x_e[t][:], AF.Identity,
                                             bias=mb, scale=ri)

                    ln_phase(x_e, ESZ, emit_ln1)
                    pack_transpose(xhs, xT, ESZ, 4, maxw=ET)

                    # QKV: Q_T/K_T feature-major, V token-major per-sample (+ones)
                    qk = [apool.tile([128, ET], BF16, tag=f"eqk{f}", name=f"eqk{f}", bufs=2)
                          for f in range(6)]
                    for fo in range(6):
                        ps = psb.tile([128, ET], F32, tag="psbig", name="psbig")
                        for kc in range(EKC):
                            nc.tensor.matmul(ps[:],
                                             qkvw_sb[:, kc, 128 * fo:128 * (fo + 1)],
                                             xT[kc][:], start=(kc == 0),
                                             stop=(kc == EKC - 1))
                        nc.scalar.activation(qk[fo][:], ps[:], AF.Copy)
                    v_sb = [apool.tile([50, HEADS * (DH + 1)], BF16, tag=f"ev{s}",
                                       name=f"ev{s}") for s in range(BS)]
                    for s in range(BS):
                        ps = psb.tile([50, D], F32, tag="psbig", name="psbig")
                        for kc in range(EKC):
                            nc.tensor.matmul(ps[:], xT[kc][:, 50 * s:50 * s + 50],
                                             qkvw_sb[:, kc, 2 * D:3 * D],
                                             start=(kc == 0), stop=(kc == EKC - 1))
                        v3 = v_sb[s][:].rearrange("p (h e) -> p h e", e=DH + 1)
                        nc.vector.tensor_copy(
                            v3[:, :, 0:DH], ps[:].rearrange("p (h e) -> p h e", e=DH))
                        nc.gpsimd.memset(v3[:, :, DH:DH + 1], 1.0)

                    # attention
                    o_sb = [apool.tile([50, D], BF16, tag=f"eo{s}", name=f"eo{s}")
                            for s in range(BS)]
                    for s in range(BS):
                        es = apool.tile([50, HEADS * 50], BF16, tag="ees", name="ees",
                                        bufs=3)
                        for h in range(HEADS):
                            kc_h, off = divmod(DH * h, 128)
                            sps = psS.tile([50, 50], F32, tag="psS", name="psS")
                            nc.tensor.matmul(
                                sps[:],
                                qk[3 + kc_h][off:off + DH, 50 * s:50 * s + 50],
                                qk[kc_h][off:off + DH, 50 * s:50 * s + 50],
                                start=True, stop=True)
                            nc.scalar.activation(es[:, 50 * h:50 * (h + 1)], sps[:],
                                                 AF.Exp)
                        for h in range(HEADS):
                            ops = psO.tile([50, DH + 1], F32, tag="psO", name="psO")
                            nc.tensor.matmul(
                                ops[:], es[:, 50 * h:50 * (h + 1)],
                                v_sb[s][:, (DH + 1) * h:(DH + 1) * (h + 1)],
                                start=True, stop=True)
                            rz = spool.tile([50, 1], F32, tag="rz", name="rz")
                            nc.vector.reciprocal(rz[:], ops[:, DH:DH + 1])
                            nc.vector.tensor_scalar_mul(
                                o_sb[s][:, DH * h:DH * (h + 1)],
                                ops[:, 0:DH], rz[:])

                    # transpose O, out-proj, residual
                    oT = [apool.tile([128, ET], BF16, tag=f"eoT{f}", name=f"eoT{f}", bufs=2)
                          for f in range(EKC)]
                    pack_transpose(o_sb, oT, [50] * BS, 8, maxw=ET)
                    for c in range(ECH):
                        ps = psb.tile([ESZ[c], D], F32, tag="psbig", name="psbig")
                        for kc in range(EKC):
                            nc.tensor.matmul(ps[:],
                                             oT[kc][:, EOFF[c]:EOFF[c] + ESZ[c]],
                                             ow_sb[:, kc, :],
                                             start=(kc == 0), stop=(kc == EKC - 1))
                        nc.vector.tensor_add(x_e[c][:], x_e[c][:], ps[:])

                    # LN2 + transpose, FC1+gelu, FC2, residual
                    x2T = [apool.tile([128, ET], BF16, tag=f"ex2T{f}", name=f"ex2T{f}", bufs=2)
                           for f in range(EKC)]
                    xh2s = [apool.tile([ESZ[c], D], BF16, tag=f"exh2{c}",
                                       name=f"exh2{c}", bufs=2) for c in range(ECH)]

                    def emit_ln2(t, mb, ri, xh2s=xh2s):
                        nc.scalar.activation(xh2s[t][:], x_e[t][:], AF.Identity,
                                             bias=mb, scale=ri)

                    ln_phase(x_e, ESZ, emit_ln2)
                    pack_transpose(xh2s, x2T, ESZ, 4, maxw=ET)
                    hsb = [apool.tile([128, ET], BF16, tag=f"eh{f}", name=f"eh{f}", bufs=2)
                           for f in range(EFH // 128)]
                    for fo in range(EFH // 128):
                        ps = psb.tile([128, ET], F32, tag="psbig", name="psbig")
                        for kc in range(EKC):
                            nc.tensor.matmul(ps[:],
                                             w1_sb[:, kc, 128 * fo:128 * (fo + 1)],
                                             x2T[kc][:], start=(kc == 0),
                                             stop=(kc == EKC - 1))
                        nc.scalar.activation(hsb[fo][:], ps[:], AF.Gelu)
                    for c in range(ECH):
                        ps = psb.tile([ESZ[c], D], F32, tag="psbig", name="psbig")
                        for kc in range(EFH // 128):
                            nc.tensor.matmul(ps[:],
                                             hsb[kc][:, EOFF[c]:EOFF[c] + ESZ[c]],
                                             w2_sb[:, kc, :],
                                             start=(kc == 0),
                                             stop=(kc == EFH // 128 - 1))
                        nc.vector.tensor_add(x_e[c][:], x_e[c][:], ps[:])

                for i in range(DEPTH):
                    enc_layer(i)

                # ==== encoder final LN + dec-embed -> decoder init ====
                for t in range(DTILES):
                    nc.gpsimd.dma_start(x_d[t][:], dpe[98 * t:98 * (t + 1), :])
                dew_sb = apool.tile([128, EKC, DD], BF16, tag="dew", name="dew")
                nc.gpsimd.dma_start(dew_sb[:],
                                  dew[:].rearrange("(kc p) f -> p kc f", p=128))
                xfT = [apool.tile([128, ET], BF16, tag=f"exT{f}", name=f"exT{f}", bufs=2)
                       for f in range(EKC)]
                xhfs = [apool.tile([ESZ[c], D], BF16, tag=f"exh{c}", name=f"exh{c}",
                                   bufs=2) for c in range(ECH)]

                def emit_lnf(t, mb, ri):
                    nc.scalar.activation(xhfs[t][:], x_e[t][:], AF.Identity,
                                         bias=mb, scale=ri)

                ln_phase(x_e, ESZ, emit_lnf)
                pack_transpose(xhfs, xfT, ESZ, 4, maxw=ET)
                for s in range(BS):
                    ps = psb.tile([NVIS, DD], F32, tag="psbig", name="psbig")
                    for kc in range(EKC):
                        nc.tensor.matmul(ps[:],
                                         xfT[kc][:, 50 * s + 1:50 * s + 1 + NVIS],
                                         dew_sb[:, kc, :],
                                         start=(kc == 0), stop=(kc == EKC - 1))
                    nc.vector.tensor_add(x_d[2 * s][0:NVIS, :],
                                         x_d[2 * s][0:NVIS, :], ps[:])

            # ================= DECODER =================
            with (
                tc.tile_pool(name="dwpool", bufs=2) as wpool,
                tc.tile_pool(name="dapool", bufs=1) as apool,
            ):
                def dec_layer(i):
                    qkvw_sb = wpool.tile([128, DKC, 3 * DD], BF16, tag="dqkvw",
                                         name="dqkvw")
                    nc.gpsimd.dma_start(qkvw_sb[:],
                                      d_qkvw[i].rearrange("(kc p) f -> p kc f", p=128))
                    ow_sb = wpool.tile([128, DKC, DD], BF16, tag="dow", name="dow")
                    nc.gpsimd.dma_start(ow_sb[:],
                                      d_ow[i].rearrange("(kc p) f -> p kc f", p=128))
                    w1_sb = wpool.tile([128, DKC, DFH], BF16, tag="dw1", name="dw1")
                    nc.gpsimd.dma_start(w1_sb[:],
                                      d_w1[i].rearrange("(kc p) f -> p kc f", p=128))
                    w2_sb = wpool.tile([128, DFH // 128, DD], BF16, tag="dw2",
                                       name="dw2")
                    nc.gpsimd.dma_start(w2_sb[:],
                                      d_w2[i].rearrange("(kc p) f -> p kc f", p=128))

                    xT = [apool.tile([128, DT], BF16, tag=f"dxT{f}", name=f"dxT{f}", bufs=2)
                          for f in range(DKC)]
                    xhs = [apool.tile([98, DD], BF16, tag=f"dxh{t}", name=f"dxh{t}")
                           for t in range(DTILES)]

                    def emit_dln1(t, mb, ri, xhs=xhs):
                        nc.scalar.activation(xhs[t][:], x_d[t][:], AF.Identity,
                                             bias=mb, scale=ri)

                    ln_phase(x_d, [98] * DTILES, emit_dln1)
                    pack_transpose(xhs, xT, [98] * DTILES, 8, maxw=784)

                    qk = [apool.tile([128, DT], BF16, tag=f"dqk{f}", name=f"dqk{f}", bufs=2)
                          for f in range(4)]
                    for fo in range(4):
                        for no in range(4):
                            ps = psb.tile([128, DNCH], F32, tag="psbig", name="psbig")
                            for kc in range(DKC):
                                nc.tensor.matmul(
                                    ps[:], qkvw_sb[:, kc, 128 * fo:128 * (fo + 1)],
                                    xT[kc][:, DNCH * no:DNCH * (no + 1)],
                                    start=(kc == 0), stop=(kc == DKC - 1))
                            nc.scalar.activation(
                                qk[fo][:, DNCH * no:DNCH * (no + 1)], ps[:], AF.Copy)
                    v_sb = [apool.tile([98, DHEADS * (DDH + 1)], BF16, tag=f"dv{t}",
                                       name=f"dv{t}") for t in range(DTILES)]
                    for t in range(DTILES):
                        ps = psb.tile([98, DD], F32, tag="psbig", name="psbig")
                        for kc in range(DKC):
                            nc.tensor.matmul(ps[:], xT[kc][:, 98 * t:98 * (t + 1)],
                                             qkvw_sb[:, kc, 2 * DD:3 * DD],
                                             start=(kc == 0), stop=(kc == DKC - 1))
                        v3 = v_sb[t][:].rearrange("p (h e) -> p h e", e=DDH + 1)
                        nc.vector.tensor_copy(
                            v3[:, :, 0:DDH],
                            ps[:].rearrange("p (h e) -> p h e", e=DDH))
                        nc.gpsimd.memset(v3[:, :, DDH:DDH + 1], 1.0)

                    o_sb = [apool.tile([98, DD], BF16, tag=f"do{t}", name=f"do{t}")
                            for t in range(DTILES)]
                    for s in range(BS):
                        for hp in range(DHEADS // 2):
                            es = []
                            for j in range(2):
                                e_j = apool.tile([98, 2 * NP], BF16, tag="des",
                                                 name="des", bufs=4)
                                for hh in range(2):
                                    h = 2 * hp + hh
                                    kc_h, off = divmod(DDH * h, 128)
                                    sps = psS.tile([98, NP], F32, tag="psS",
                                                   name="psS")
                                    nc.tensor.matmul(
                                        sps[:],
                                        qk[2 + kc_h][off:off + DDH,
                                                     NP * s + 98 * j:NP * s + 98 * (j + 1)],
                                        qk[kc_h][off:off + DDH, NP * s:NP * (s + 1)],
                                        start=True, stop=True,
                                        tile_position=(off, 0))
                                    nc.scalar.activation(
                                        e_j[:, NP * hh:NP * (hh + 1)], sps[:], AF.Exp)
                                es.append(e_j)
                            for hh in range(2):
                                h = 2 * hp + hh
                                for qc in range(2):
                                    ops = psO.tile([98, DDH + 1], F32, tag="psO",
                                                   name="psO")
                                    for j in range(2):
                                        nc.tensor.matmul(
                                            ops[:],
                                            es[j][:, NP * hh + 98 * qc:
                                                  NP * hh + 98 * (qc + 1)],
                                            v_sb[2 * s + j][:,
                                                            (DDH + 1) * h:(DDH + 1) * (h + 1)],
                                            start=(j == 0), stop=(j == 1))
                                    rz = spool.tile([98, 1], F32, tag="rz", name="rz")
                                    nc.vector.reciprocal(rz[:], ops[:, DDH:DDH + 1])
                                    nc.vector.tensor_scalar_mul(
                                        o_sb[2 * s + qc][:, DDH * h:DDH * (h + 1)],
                                        ops[:, 0:DDH], rz[:])

                    oT = [apool.tile([128, DT], BF16, tag=f"doT{f}", name=f"doT{f}", bufs=2)
                          for f in range(DKC)]
                    pack_transpose(o_sb, oT, [98] * DTILES, 8, maxw=784)
                    for t in range(DTILES):
                        ps = psb.tile([98, DD], F32, tag="psbig", name="psbig")
                        for kc in range(DKC):
                            nc.tensor.matmul(ps[:], oT[kc][:, 98 * t:98 * (t + 1)],
                                             ow_sb[:, kc, :],
                                             start=(kc == 0), stop=(kc == DKC - 1))
                        nc.vector.tensor_add(x_d[t][:], x_d[t][:], ps[:])

                    x2T = [apool.tile([128, DT], BF16, tag=f"dx2T{f}",
                                      name=f"dx2T{f}") for f in range(DKC)]
                    xh2s = [apool.tile([98, DD], BF16, tag=f"dxh2{t}",
                                       name=f"dxh2{t}") for t in range(DTILES)]

                    def emit_dln2(t, mb, ri, xh2s=xh2s):
                        nc.scalar.activation(xh2s[t][:], x_d[t][:], AF.Identity,
                                             bias=mb, scale=ri)

                    ln_phase(x_d, [98] * DTILES, emit_dln2)
                    pack_transpose(xh2s, x2T, [98] * DTILES, 8, maxw=784)
                    hsb = [apool.tile([128, DT], BF16, tag=f"dh{f}", name=f"dh{f}")
                           for f in range(DFH // 128)]
                    for fo in range(DFH // 128):
                        for no in range(4):
                            ps = psb.tile([128, DNCH], F32, tag="psbig", name="psbig")
                            for kc in range(DKC):
                                nc.tensor.matmul(
                                    ps[:], w1_sb[:, kc, 128 * fo:128 * (fo + 1)],
                                    x2T[kc][:, DNCH * no:DNCH * (no + 1)],
                                    start=(kc == 0), stop=(kc == DKC - 1))
                            nc.scalar.activation(
                                hsb[fo][:, DNCH * no:DNCH * (no + 1)], ps[:], AF.Gelu)
                    for t in range(DTILES):
                        ps = psb.tile([98, DD], F32, tag="psbig", name="psbig")
                        for kc in range(DFH // 128):
                            nc.tensor.matmul(ps[:], hsb[kc][:, 98 * t:98 * (t + 1)],
                                             w2_sb[:, kc, :],
                                             start=(kc == 0),
                                             stop=(kc == DFH // 128 - 1))
                        nc.vector.tensor_add(x_d[t][:], x_d[t][:], ps[:])

                for i in range(DDEPTH):
                    dec_layer(i)

                # ======== final LN + pred head ========
                pw_sb = apool.tile([128, DKC, P * P * C], BF16, tag="pw", name="pw")
                nc.gpsimd.dma_start(pw_sb[:],
                                  predw[:].rearrange("(kc p) f -> p kc f", p=128))
                xdT = [apool.tile([128, DT], BF16, tag=f"dxT{f}", name=f"dxT{f}", bufs=2)
                       for f in range(DKC)]
                xhds = [apool.tile([98, DD], BF16, tag=f"dxh{t}", name=f"dxh{t}")
                        for t in range(DTILES)]

                def emit_dlnf(t, mb, ri):
                    nc.scalar.activation(xhds[t][:], x_d[t][:], AF.Identity,
                                         bias=mb, scale=ri)

                ln_phase(x_d, [98] * DTILES, emit_dlnf)
                pack_transpose(xhds, xdT, [98] * DTILES, 8, maxw=784)
                for t in range(DTILES):
                    osb = apool.tile([98, P * P * C], F32, tag="outsb", name="outsb",
                                     bufs=2)
                    for no in range(2):
                        ps = psb.tile([98, 384], F32, tag="psbig", name="psbig")
                        for kc in range(DKC):
                            nc.tensor.matmul(ps[:], xdT[kc][:, 98 * t:98 * (t + 1)],
                                             pw_sb[:, kc, 384 * no:384 * (no + 1)],
                                             start=(kc == 0), stop=(kc == DKC - 1))
                        nc.vector.tensor_copy(osb[:, 384 * no:384 * (no + 1)], ps[:])
                    nc.sync.dma_start(out[98 * t:98 * (t + 1), :], osb[:])

    nc.finalize()
    return nc


def _host_prep(inputs):
    ui = np.asarray(inputs["unmasked_idx"])
    mi = np.asarray(inputs["masked_idx"])
    perm = np.concatenate([ui, mi], axis=1)               # [B, 196]
    x = _f32(inputs["x"])
    patches = x.reshape(B, C, 14, P, 14, P).transpose(0, 2, 4, 1, 3, 5) \
               .reshape(B, NP, PATCH)
    bi = np.arange(B)[:, None]
    xg = patches[bi, ui]                                  # [B, 49, 768]
    xg50 = np.concatenate([np.zeros((B, 1, PATCH), np.float32), xg], axis=1)

    pe = _f32(inputs["pos_embed"])
    bias_tok = np.zeros((B, 50, D), np.float32)
    bias_tok[:, 0] = _f32(inputs["cls_token"]) + pe[0]
    bias_tok[:, 1:50] = _f32(inputs["conv_b"]) + pe[ui + 1]

    # decoder positional stream in permuted order, with mask_token and
    # dec-embed bias folded in
    dde = _f32(inputs["dec_pos_embed"])
    bde = _f32(inputs["dec_embed_b"]) + _f32(inputs["dec_embed_w"]) @ _f32(inputs["enc_norm_b"])
    dpe = dde[perm + 1].copy()                            # [B, 196, 256]
    dpe[:, NVIS:] += _f32(inputs["mask_token"])
    dpe[:, :NVIS] += bde

    def fold(pfx, depth, d, dh):
        qkvw_t = np.empty((depth, d, 3 * d), np.float32)
        ow_t = np.empty((depth, d, d), np.float32)
        w1_t = np.empty((depth, d, 4 * d), np.float32)
        w2_t = np.empty((depth, 4 * d, d), np.float32)
        for i in range(depth):
            Wq = _f32(inputs[f"{pfx}_qkv_w"][i]) * _f32(inputs[f"{pfx}_ln1_w"][i])[None, :]
            Wq[:d] /= np.sqrt(dh)
            qkvw_t[i] = Wq.T
            ow_t[i] = _f32(inputs[f"{pfx}_out_w"][i]).T
            W1 = _f32(inputs[f"{pfx}_fc1_w"][i]) * _f32(inputs[f"{pfx}_ln2_w"][i])[None, :]
            w1_t[i] = W1.T
            w2_t[i] = _f32(inputs[f"{pfx}_fc2_w"][i]).T
        return qkvw_t, ow_t, w1_t, w2_t

    e_qkvw, e_ow, e_w1, e_w2 = fold("enc", DEPTH, D, DH)
    d_qkvw, d_ow, d_w1, d_w2 = fold("dec", DDEPTH, DD, DDH)

    # biases are structurally zero in this model (see spec fills); the LN
    # biases fold into the arrays above. Guard so silent wrongness is
    # impossible if that ever changes.
    for k in ("conv_b", "enc_qkv_b", "enc_out_b", "enc_fc1_b", "enc_fc2_b",
              "dec_qkv_b", "dec_out_b", "dec_fc1_b", "dec_fc2_b",
              "enc_ln1_b", "enc_ln2_b", "dec_ln1_b", "dec_ln2_b",
              "enc_norm_b", "dec_norm_b", "dec_embed_b", "pred_b"):
        if k in ("conv_b",):  # folded into bias_tok already
            continue
        assert np.max(np.abs(_f32(inputs[k]))) == 0.0, f"nonzero bias {k} unsupported"

    dew = (_f32(inputs["dec_embed_w"]) * _f32(inputs["enc_norm_w"])[None, :]).T
    predw = (_f32(inputs["pred_w"]) * _f32(inputs["dec_norm_w"])[None, :]).T
    bp = _f32(inputs["pred_b"]) + _f32(inputs["pred_w"]) @ _f32(inputs["dec_norm_b"])
    wconv_t = _f32(inputs["conv_w"]).reshape(D, PATCH).T

    shared = {
        "wconv_t": _bf(wconv_t),
        "e_qkvw": _bf(e_qkvw), "e_ow": _bf(e_ow),
        "e_w1": _bf(e_w1), "e_w2": _bf(e_w2),
        "d_qkvw": _bf(d_qkvw), "d_ow": _bf(d_ow),
        "d_w1": _bf(d_w1), "d_w2": _bf(d_w2),
        "dew": _bf(dew), "predw": _bf(predw),
    }
    in_maps = []
    for c in range(NCORES):
        sl = slice(c * BS, (c + 1) * BS)
        m = dict(shared)
        m["xg_t"] = _bf(xg50[sl].reshape(BS * 50, PATCH).T)
        m["bias_tok"] = _f32(bias_tok[sl].reshape(ET, D))
        m["dpe"] = _f32(dpe[sl].reshape(DT, DD))
        in_maps.append(m)
    return in_maps, perm, bp


def kernel(**inputs):
    if "nc" not in _CACHE:
        _CACHE["nc"] = _build_program()
    nc = _CACHE["nc"]
    in_maps, perm, bp = _host_prep(inputs)
    res = run_bass_kernel_spmd(nc, in_maps, list(range(NCORES)))
    _CACHE["last_res"] = res
    dev = np.stack([np.asarray(res.results[c]["out"], np.float32).reshape(BS, NP, P * P * C)
                    for c in range(NCORES)]).reshape(B, NP, P * P * C)
    out = np.empty((B, NP, P * P * C), np.float32)
    out[np.arange(B)[:, None], perm] = dev + bp
    return out

